# revision 27
# baseline (speedup 1.0000x reference)
"""Causal multi-head attention on 8 trn2 NeuronCores (v2: bf16, SBUF-resident).

Problem: B=4, S=2048, D=2048, H=16 heads, head_dim=128, causal softmax,
torch-style Linear projections (W stored [in, out]).

Sharding: core c handles batch b = c//2 and head-group g = c%2
(8 heads = 1024 output columns of Wq/Wk/Wv, 1024 rows of Wo).
Each core produces a partial output [S, D] (bf16); host sums the two
head-group partials per batch and adds bo.

v2 design (vs the fp32r/DRAM-scratch v1):
  - All matmul operands bf16: fast-weight-load hides LDWEIGHTS, fixes the
    fp32r small-N penalty, halves DMA. Accumulation stays f32 in PSUM.
  - Q^T, K^T, V, ct all SBUF-resident (no DRAM scratch round-trips).
  - Softmax denominator via a DVE add-tree over exp tiles plus ONE
    ones-matmul per (head, q-chunk) instead of a ones-matmul per k-tile:
    removes ~50us of PE work.
  - qc-major phase B with the output projection (phase C) of chunk qc
    emitted after phase B of chunk qc+1 so PE never waits on the softmax
    normalization tail.
"""

import numpy as np
import ml_dtypes

import concourse.bass as bass
import concourse.mybir as mybir
import concourse.tile as tile
from concourse import bacc
from concourse.bass_utils import run_bass_kernel_spmd

B = 4
S = 2048
D = 2048
H = 16
DH = 128
HPC = 8          # heads per core
DHG = HPC * DH   # 1024: head-group width per core
KT = D // 128    # 16 k-tiles over the model dim
ST = S // 128    # 16 s-tiles
QC = S // 512    # 4 q-chunks
SCALE = 1.0 / np.sqrt(DH)
NEG = -1.0e30

F32 = mybir.dt.float32
BF16 = mybir.dt.bfloat16
BF = ml_dtypes.bfloat16

EXP = mybir.ActivationFunctionType.Exp
IDENT = mybir.ActivationFunctionType.Identity
COPY = mybir.ActivationFunctionType.Copy
ADD = mybir.AluOpType.add
MULT = mybir.AluOpType.mult


def _build_nc():
    nc = bacc.Bacc(None, target_bir_lowering=False)

    xT = nc.declare_dram_parameter("xT", [D, S], BF16, isOutput=False)
    # wq/wk host-pregathered to [HPC*128, KT*128]: row t*128+p, col n*128+m
    # = Wq[n*128+p, g*1024 + t*128+m] so each head-tile's weights DMA
    # contiguously as [128, KT, 128]
    wq = nc.declare_dram_parameter("wq", [DHG, D], BF16, isOutput=False)
    wk = nc.declare_dram_parameter("wk", [DHG, D], BF16, isOutput=False)
    wv = nc.declare_dram_parameter("wv", [D, DHG], BF16, isOutput=False)
    wo = nc.declare_dram_parameter("wo", [DHG, D], BF16, isOutput=False)
    bqT = nc.declare_dram_parameter("bqT", [128, HPC], F32, isOutput=False)
    bkT = nc.declare_dram_parameter("bkT", [128, HPC], F32, isOutput=False)
    cmask = nc.declare_dram_parameter("cmask", [128, 896], F32, isOutput=False)
    out = nc.declare_dram_parameter("out", [S, D], BF16, isOutput=True)

    with tile.TileContext(nc) as tc:
        _emit(nc, tc, xT, wq, wk, wv, wo, bqT, bkT, cmask, out)
    nc.compile()
    return nc


def _emit(nc, tc, xT, wq, wk, wv, wo, bqT, bkT, cmask, out):
    with (
        tc.tile_pool(name="qt", bufs=1) as qtp,
        tc.tile_pool(name="kt", bufs=1) as ktp,
        tc.tile_pool(name="v", bufs=1) as vp,
    ):
        qt_all = qtp.tile([128, HPC, S], BF16)   # [head dim, head, s]
        kt_all = ktp.tile([128, HPC, S], BF16)
        v_all = vp.tile([128, ST, DHG], BF16)    # [s in tile, s-tile, head*dh]

        # ---------------- Phase A: projections into SBUF --------------------
        with (
            tc.tile_pool(name="xa", bufs=1) as xap,
            tc.tile_pool(name="wvp", bufs=1) as wvp,
            tc.tile_pool(name="wqk", bufs=3) as wqk,
            tc.tile_pool(name="abias", bufs=1) as abp,
            tc.tile_pool(name="apsum", bufs=8, space="PSUM") as aps,
        ):
            # DMA queue split: weight tiles on sync, x on gpsimd, the rest on
            # scalar — so the first matmul's operands aren't queued behind
            # 12 MiB of bulk input
            bq_sb = abp.tile([128, HPC], F32)
            nc.scalar.dma_start(out=bq_sb, in_=bqT[:, :])
            bk_sb = abp.tile([128, HPC], F32)
            nc.scalar.dma_start(out=bk_sb, in_=bkT[:, :])

            # Q^T / K^T: per (w, head-tile): psum[dh 128, s 512] over 16 kd
            seq = [(w, b, dst, t)
                   for t in range(HPC)
                   for w, b, dst in ((wq, bq_sb, qt_all), (wk, bk_sb, kt_all))]
            w_tiles = {}

            def w_prefetch(i):
                if i < len(seq):
                    w, _, _, t = seq[i]
                    w_sb = wqk.tile([128, KT, 128], BF16, tag="wqk",
                                    name=f"w_sb{i % 3}")
                    nc.sync.dma_start(
                        out=w_sb,
                        in_=w[t * 128 : (t + 1) * 128, :]
                        .rearrange("p (n m) -> p n m", m=128),
                    )
                    w_tiles[i] = w_sb

            w_prefetch(0)
            w_prefetch(1)
            w_prefetch(2)

            # x split across the gpsimd and sync queues (the first three
            # weight tiles are already ahead of it on sync)
            x_all = xap.tile([128, KT, S], BF16)
            for kd in range(KT):
                eng = nc.gpsimd if kd % 2 == 0 else nc.sync
                eng.dma_start(
                    out=x_all[:, kd, :], in_=xT[kd * 128 : (kd + 1) * 128, :]
                )
            wv_sb = wvp.tile([128, KT, DHG], BF16)
            wv_r = wv.rearrange("(n p) m -> p n m", p=128)
            nc.scalar.dma_start(out=wv_sb[:, 0:8, :], in_=wv_r[:, 0:8, :])
            nc.scalar.dma_start(out=wv_sb[:, 8:16, :], in_=wv_r[:, 8:16, :])

            def qk_stage(dst, t, b_sb, psums):
                for sc in range(4):
                    nc.scalar.activation(
                        out=dst[:, t, sc * 512 : (sc + 1) * 512],
                        in_=psums[sc],
                        func=IDENT,
                        bias=b_sb[:, t : t + 1],
                    )

            # groups 0+1 interleaved kd-wise (8 psum banks): during the x
            # stream the PE has 8 matmuls ready per arrived kd slice, so it is
            # never DMA-paced
            pair = []
            for i in (0, 1):
                w, b_sb, dst, t = seq[i]
                psums = [aps.tile([128, 512], F32, tag="apsum", name=f"qk{i}{sc}")
                         for sc in range(4)]
                pair.append((w_tiles.pop(i), b_sb, dst, t, psums))
            for kd in range(KT):
                for w_sb, _, _, _, psums in pair:
                    for sc in range(4):
                        nc.tensor.matmul(
                            psums[sc],
                            w_sb[:, kd, :],
                            x_all[:, kd, sc * 512 : (sc + 1) * 512],
                            start=(kd == 0),
                            stop=(kd == KT - 1),
                        )
            for _, b_sb, dst, t, psums in pair:
                qk_stage(dst, t, b_sb, psums)
            w_prefetch(3)
            w_prefetch(4)

            for i, (w, b_sb, dst, t) in enumerate(seq):
                if i < 2:
                    continue
                w_sb = w_tiles.pop(i)
                w_prefetch(i + 3)
                psums = [aps.tile([128, 512], F32, tag="apsum", name=f"qk{sc}")
                         for sc in range(4)]
                for kd in range(KT):
                    for sc in range(4):
                        nc.tensor.matmul(
                            psums[sc],
                            w_sb[:, kd, :],
                            x_all[:, kd, sc * 512 : (sc + 1) * 512],
                            start=(kd == 0),
                            stop=(kd == KT - 1),
                        )
                qk_stage(dst, t, b_sb, psums)

            # V: per s-tile: psum[s 128, dh 512] x2 halves over 16 kd
            for si in range(ST):
                ps = [aps.tile([128, 512], F32, tag="apsum", name=f"v{si % 2}{half}")
                      for half in range(2)]
                for kd in range(KT):
                    for half in range(2):
                        nc.tensor.matmul(
                            ps[half],
                            x_all[:, kd, si * 128 : (si + 1) * 128],
                            wv_sb[:, kd, half * 512 : (half + 1) * 512],
                            start=(kd == 0),
                            stop=(kd == KT - 1),
                        )
                # no bias: P@(V+1(x)bv) folds to +bv@Wo, added host-side to bo
                for half in range(2):
                    nc.scalar.activation(
                        out=v_all[:, si, half * 512 : (half + 1) * 512],
                        in_=ps[half],
                        func=COPY,
                    )

        # ---------------- Phase B + C interleaved ----------------------------
        with (
            tc.tile_pool(name="wop", bufs=1) as wop,
            tc.tile_pool(name="bconst", bufs=1) as bcp,
            tc.tile_pool(name="ct", bufs=4) as ctpool,
            tc.tile_pool(name="ptile", bufs=6) as ppool,
            tc.tile_pool(name="msk", bufs=4) as mpool,
            tc.tile_pool(name="den", bufs=4) as dpool,
            tc.tile_pool(name="rcp", bufs=2) as rcpool,
            tc.tile_pool(name="ostage", bufs=4) as ost,
            tc.tile_pool(name="pscore", bufs=4, space="PSUM") as pscore,
            tc.tile_pool(name="pctx", bufs=2, space="PSUM") as pctx,
            tc.tile_pool(name="opsum", bufs=2, space="PSUM") as ops,
        ):
            cm_sb = bcp.tile([128, 896], F32)
            nc.sync.dma_start(out=cm_sb, in_=cmask[:, :])
            ones_bf = bcp.tile([128, 128], BF16)
            nc.vector.memset(ones_bf, 1.0)
            wo_sb = wop.tile([128, HPC, D], BF16)
            wo_r = wo.rearrange("(n p) m -> p n m", p=128)
            nc.sync.dma_start(out=wo_sb[:, 0:4, :], in_=wo_r[:, 0:4, :])
            nc.scalar.dma_start(out=wo_sb[:, 4:8, :], in_=wo_r[:, 4:8, :])

            ct_chunks = {}

            def phase_b(qc):
                nkt = 4 * qc + 4
                ct_chunk = ctpool.tile([128, HPC, 512], BF16, tag="ct",
                                       name=f"ct{qc % 2}")
                ct_chunks[qc] = ct_chunk
                for h in range(HPC):
                    # diagonal tiles first: longer PE->DVE->ACT chains start
                    # early and overlap with the full tiles' stream
                    order = list(range(4 * qc, nkt)) + list(range(4 * qc))
                    psum_c = pctx.tile([128, 512], F32, tag="pctx")
                    hd = {}

                    def scores(kt_i):
                        j = kt_i - 4 * qc
                        off = 128 * j if j > 0 else 0
                        ps_t = pscore.tile([128, 512], F32, tag="ps_t")
                        nc.tensor.matmul(
                            ps_t[:, off:],
                            kt_all[:, h, kt_i * 128 : (kt_i + 1) * 128],
                            qt_all[:, h, qc * 512 + off : (qc + 1) * 512],
                            start=True,
                            stop=True,
                        )
                        p_t = ppool.tile([128, 512], BF16, tag="p_t")
                        if j >= 0:
                            msk = mpool.tile([128, 512], F32, tag="msk")
                            nc.vector.tensor_tensor(
                                out=msk[:, off:],
                                in0=ps_t[:, off:],
                                in1=cm_sb[:, 384 : 896 - off],
                                op=ADD,
                            )
                            src = msk
                        else:
                            src = ps_t
                        nc.scalar.activation(
                            out=p_t[:, off:],
                            in_=src[:, off:],
                            func=EXP,
                            scale=float(SCALE),
                        )
                        # denominator accumulation on DVE. j==0's tile is kept
                        # pristine; j==1 initializes the accumulator from it in
                        # two pieces (cols 0:128 only ever see the j==0 tile)
                        if j == 0:
                            hd["d0"] = p_t
                        elif j == 1:
                            den = dpool.tile([128, 512], BF16, tag="den")
                            hd["den"] = den
                            nc.vector.tensor_copy(
                                out=den[:, 0:128], in_=hd["d0"][:, 0:128]
                            )
                            nc.vector.tensor_tensor(
                                out=den[:, 128:], in0=hd["d0"][:, 128:],
                                in1=p_t[:, 128:], op=ADD,
                            )
                        elif j > 1:
                            den = hd["den"]
                            nc.vector.tensor_tensor(
                                out=den[:, off:], in0=den[:, off:],
                                in1=p_t[:, off:], op=ADD,
                            )
                        else:
                            # full tiles: two interleaved chains so the
                            # head-end latency is halved
                            nf = hd.get("nf", 0)
                            hd["nf"] = nf + 1
                            if nf == 0:
                                hd["pf0"] = p_t
                            elif nf == 1:
                                den2 = dpool.tile([128, 512], BF16, tag="den",
                                                  name="den2")
                                hd["den2"] = den2
                                nc.vector.tensor_tensor(
                                    out=den2, in0=hd["pf0"], in1=p_t, op=ADD,
                                )
                            else:
                                tgt = hd["den"] if nf % 2 == 0 else hd["den2"]
                                nc.vector.tensor_tensor(
                                    out=tgt, in0=tgt, in1=p_t, op=ADD,
                                )
                        return p_t, off

                    def ctx(idx, kt_i, p_t, off):
                        nc.tensor.matmul(
                            psum_c[:, off:],
                            v_all[:, kt_i, h * 128 : (h + 1) * 128],
                            p_t[:, off:],
                            start=(idx == 0),
                            stop=(idx == nkt - 1),
                        )

                    # software-pipeline scores/exp ahead of ctx by two tiles
                    # so the PE never waits on the DVE-mask -> ACT-exp chain
                    pend = []
                    ctx_i = 0
                    for kt_i in order:
                        p_t, off = scores(kt_i)
                        pend.append((kt_i, p_t, off))
                        if len(pend) > 2:
                            ctx(ctx_i, *pend.pop(0))
                            ctx_i += 1
                    for ent in pend:
                        ctx(ctx_i, *ent)
                        ctx_i += 1

                    if "den2" in hd:
                        nc.vector.tensor_tensor(
                            out=hd["den"], in0=hd["den"], in1=hd["den2"], op=ADD,
                        )
                    psum_den = pscore.tile([128, 512], F32, tag="ps_t",
                                           name="psum_den")
                    nc.tensor.matmul(
                        psum_den, ones_bf, hd["den"], start=True, stop=True
                    )
                    recip = rcpool.tile([128, 512], F32, tag="rcp")
                    nc.vector.reciprocal_approx_fast(out=recip, in_=psum_den)
                    nc.vector.tensor_tensor(
                        out=ct_chunk[:, h, :],
                        in0=psum_c,
                        in1=recip,
                        op=MULT,
                    )

            def phase_c(qc):
                ct_chunk = ct_chunks.pop(qc)
                for st4 in range(4):
                    st = qc * 4 + st4
                    for ncol in range(4):
                        psum_o = ops.tile([128, 512], F32, tag="opsum")
                        for hh in range(HPC):
                            nc.tensor.matmul(
                                psum_o,
                                ct_chunk[:, hh, st4 * 128 : (st4 + 1) * 128],
                                wo_sb[:, hh, ncol * 512 : (ncol + 1) * 512],
                                start=(hh == 0),
                                stop=(hh == HPC - 1),
                            )
                        o_sb = ost.tile([128, 512], BF16, tag="ostage")
                        if (st4 + ncol) % 2 == 0:
                            nc.vector.tensor_copy(out=o_sb, in_=psum_o)
                        else:
                            nc.scalar.activation(out=o_sb, in_=psum_o, func=COPY)
                        nc.gpsimd.dma_start(
                            out=out[
                                st * 128 : (st + 1) * 128,
                                ncol * 512 : (ncol + 1) * 512,
                            ],
                            in_=o_sb,
                        )

            # emission order: C(qc) lands 2 chunks later so the PE queue stays
            # full while the DVE grinds the mask/denominator chains of the
            # small early chunks
            phase_b(0)
            phase_b(1)
            phase_b(2)
            phase_c(0)
            phase_b(3)
            phase_c(1)
            phase_c(2)
            phase_c(3)


_NC = None


def _get_nc():
    global _NC
    if _NC is None:
        _NC = _build_nc()
    return _NC


def _host_prep(input_sequences, Wq, bq, Wk, bk, Wv, bv, Wo, bo):
    """Build per-core input maps (all bf16 except biases/mask)."""
    x = np.asarray(input_sequences, dtype=np.float32)
    cm = np.full((128, 896), NEG, dtype=np.float32)
    kk = np.arange(128)[:, None]
    uu = np.arange(896)[None, :]
    cm[kk <= uu - 384] = 0.0

    xT_b = [np.ascontiguousarray(x[b].T).astype(BF) for b in range(B)]
    halves = []
    for g in range(2):
        sl = slice(g * DHG, (g + 1) * DHG)
        wq_c = np.ascontiguousarray(
            np.asarray(Wq[:, sl], dtype=np.float32)
            .reshape(KT, 128, HPC, 128).transpose(2, 1, 0, 3).reshape(DHG, D)
        ).astype(BF)
        wk_c = np.ascontiguousarray(
            np.asarray(Wk[:, sl], dtype=np.float32)
            .reshape(KT, 128, HPC, 128).transpose(2, 1, 0, 3).reshape(DHG, D)
        ).astype(BF)
        wv_c = np.ascontiguousarray(Wv[:, sl]).astype(BF)
        wo_c = np.ascontiguousarray(Wo[sl, :]).astype(BF)
        halves.append({
            "wq": wq_c,
            "wk": wk_c,
            "wv": wv_c,
            "wo": wo_c,
            "bqT": np.ascontiguousarray(
                np.asarray(bq[sl], dtype=np.float32).reshape(HPC, 128).T
            ),
            "bkT": np.ascontiguousarray(
                np.asarray(bk[sl], dtype=np.float32).reshape(HPC, 128).T
            ),
            "cmask": cm,
        })

    in_maps = []
    for c in range(8):
        b, g = divmod(c, 2)
        in_maps.append({"xT": xT_b[b], **halves[g]})
    return in_maps


def kernel(input_sequences, Wq, bq, Wk, bk, Wv, bv, Wo, bo, _trace=False):
    nc = _get_nc()
    in_maps = _host_prep(input_sequences, Wq, bq, Wk, bk, Wv, bv, Wo, bo)
    res = run_bass_kernel_spmd(nc, in_maps, list(range(8)), trace=_trace)
    # the V bias is folded out of the device kernel: softmax weights sum to 1,
    # so dropping bv from V shifts every output row by exactly bv @ Wo
    bo_eff = (
        np.asarray(bo, dtype=np.float32)
        + np.asarray(bv, dtype=np.float32) @ np.asarray(Wo, dtype=np.float32)
    )
    out = np.empty((B, S, D), dtype=np.float32)
    for b in range(B):
        out[b] = (
            res.results[2 * b]["out"].astype(np.float32)
            + res.results[2 * b + 1]["out"].astype(np.float32)
            + bo_eff
        )
    if _trace:
        kernel.last_exec_time_ns = res.exec_time_ns
    return out


# revision 28
# speedup vs baseline: 1.2740x; 1.2740x over previous
"""Causal multi-head attention on 8 trn2 NeuronCores (v2: bf16, SBUF-resident).

Problem: B=4, S=2048, D=2048, H=16 heads, head_dim=128, causal softmax,
torch-style Linear projections (W stored [in, out]).

Sharding: core c handles batch b = c//2 and head-group g = c%2
(8 heads = 1024 output columns of Wq/Wk/Wv, 1024 rows of Wo).
Each core produces a partial output [S, D] (bf16); host sums the two
head-group partials per batch and adds bo.

v2 design (vs the fp32r/DRAM-scratch v1):
  - All matmul operands bf16: fast-weight-load hides LDWEIGHTS, fixes the
    fp32r small-N penalty, halves DMA. Accumulation stays f32 in PSUM.
  - Q^T, K^T, V, ct all SBUF-resident (no DRAM scratch round-trips).
  - Softmax denominator via a DVE add-tree over exp tiles plus ONE
    ones-matmul per (head, q-chunk) instead of a ones-matmul per k-tile:
    removes ~50us of PE work.
  - qc-major phase B with the output projection (phase C) of chunk qc
    emitted after phase B of chunk qc+1 so PE never waits on the softmax
    normalization tail.
"""

import numpy as np
import ml_dtypes

import concourse.bass as bass
import concourse.mybir as mybir
import concourse.tile as tile
from concourse import bacc
from concourse.bass_utils import run_bass_kernel_spmd

B = 4
S = 2048
D = 2048
H = 16
DH = 128
HPC = 8          # heads per core
DHG = HPC * DH   # 1024: head-group width per core
KT = D // 128    # 16 k-tiles over the model dim
ST = S // 128    # 16 s-tiles
QC = S // 512    # 4 q-chunks
SCALE = 1.0 / np.sqrt(DH)
NEG = -1.0e30

F32 = mybir.dt.float32
BF16 = mybir.dt.bfloat16
BF = ml_dtypes.bfloat16

EXP = mybir.ActivationFunctionType.Exp
IDENT = mybir.ActivationFunctionType.Identity
COPY = mybir.ActivationFunctionType.Copy
ADD = mybir.AluOpType.add
MULT = mybir.AluOpType.mult


def _build_nc():
    nc = bacc.Bacc(None, target_bir_lowering=False)

    xT = nc.declare_dram_parameter("xT", [D, S], BF16, isOutput=False)
    # wq/wk host-pregathered to [HPC*128, KT*128]: row t*128+p, col n*128+m
    # = Wq[n*128+p, g*1024 + t*128+m] so each head-tile's weights DMA
    # contiguously as [128, KT, 128]
    wq = nc.declare_dram_parameter("wq", [DHG, D], BF16, isOutput=False)
    wk = nc.declare_dram_parameter("wk", [DHG, D], BF16, isOutput=False)
    wv = nc.declare_dram_parameter("wv", [D, DHG], BF16, isOutput=False)
    wo = nc.declare_dram_parameter("wo", [DHG, D], BF16, isOutput=False)
    bqT = nc.declare_dram_parameter("bqT", [128, HPC], F32, isOutput=False)
    bkT = nc.declare_dram_parameter("bkT", [128, HPC], F32, isOutput=False)
    cmask = nc.declare_dram_parameter("cmask", [128, 896], F32, isOutput=False)
    out = nc.declare_dram_parameter("out", [S, D], BF16, isOutput=True)

    with tile.TileContext(nc) as tc:
        _emit(nc, tc, xT, wq, wk, wv, wo, bqT, bkT, cmask, out)
    nc.compile()
    return nc


def _emit(nc, tc, xT, wq, wk, wv, wo, bqT, bkT, cmask, out):
    with (
        tc.tile_pool(name="qt", bufs=1) as qtp,
        tc.tile_pool(name="kt", bufs=1) as ktp,
        tc.tile_pool(name="v", bufs=1) as vp,
    ):
        qt_all = qtp.tile([128, HPC, S], BF16)   # [head dim, head, s]
        kt_all = ktp.tile([128, HPC, S], BF16)
        v_all = vp.tile([128, ST, DHG], BF16)    # [s in tile, s-tile, head*dh]

        # ---------------- Phase A: projections into SBUF --------------------
        with (
            tc.tile_pool(name="xa", bufs=1) as xap,
            tc.tile_pool(name="wvp", bufs=1) as wvp,
            tc.tile_pool(name="wqk", bufs=3) as wqk,
            tc.tile_pool(name="abias", bufs=1) as abp,
            tc.tile_pool(name="apsum", bufs=8, space="PSUM") as aps,
        ):
            # DMA queue split: weight tiles on sync, x on gpsimd, the rest on
            # scalar — so the first matmul's operands aren't queued behind
            # 12 MiB of bulk input
            bq_sb = abp.tile([128, HPC], F32)
            nc.scalar.dma_start(out=bq_sb, in_=bqT[:, :])
            bk_sb = abp.tile([128, HPC], F32)
            nc.scalar.dma_start(out=bk_sb, in_=bkT[:, :])

            # Q^T / K^T: per (w, head-tile): psum[dh 128, s 512] over 16 kd
            seq = [(w, b, dst, t)
                   for t in range(HPC)
                   for w, b, dst in ((wq, bq_sb, qt_all), (wk, bk_sb, kt_all))]
            w_tiles = {}

            def w_prefetch(i):
                if i < len(seq):
                    w, _, _, t = seq[i]
                    w_sb = wqk.tile([128, KT, 128], BF16, tag="wqk",
                                    name=f"w_sb{i % 3}")
                    nc.sync.dma_start(
                        out=w_sb,
                        in_=w[t * 128 : (t + 1) * 128, :]
                        .rearrange("p (n m) -> p n m", m=128),
                    )
                    w_tiles[i] = w_sb

            w_prefetch(0)
            w_prefetch(1)
            w_prefetch(2)

            # x split across the gpsimd and sync queues (the first three
            # weight tiles are already ahead of it on sync)
            x_all = xap.tile([128, KT, S], BF16)
            for kd in range(KT):
                nc.gpsimd.dma_start(
                    out=x_all[:, kd, :], in_=xT[kd * 128 : (kd + 1) * 128, :]
                )
            wv_sb = wvp.tile([128, KT, DHG], BF16)
            wv_r = wv.rearrange("(n p) m -> p n m", p=128)
            nc.scalar.dma_start(out=wv_sb[:, 0:8, :], in_=wv_r[:, 0:8, :])
            nc.scalar.dma_start(out=wv_sb[:, 8:16, :], in_=wv_r[:, 8:16, :])

            def qk_stage(dst, t, b_sb, psums):
                for sc in range(4):
                    nc.scalar.activation(
                        out=dst[:, t, sc * 512 : (sc + 1) * 512],
                        in_=psums[sc],
                        func=IDENT,
                        bias=b_sb[:, t : t + 1],
                    )

            # groups 0+1 interleaved kd-wise (8 psum banks): during the x
            # stream the PE has 8 matmuls ready per arrived kd slice, so it is
            # never DMA-paced
            pair = []
            for i in (0, 1):
                w, b_sb, dst, t = seq[i]
                psums = [aps.tile([128, 512], F32, tag="apsum", name=f"qk{i}{sc}")
                         for sc in range(4)]
                pair.append((w_tiles.pop(i), b_sb, dst, t, psums))
            for kd in range(KT):
                for w_sb, _, _, _, psums in pair:
                    for sc in range(4):
                        nc.tensor.matmul(
                            psums[sc],
                            w_sb[:, kd, :],
                            x_all[:, kd, sc * 512 : (sc + 1) * 512],
                            start=(kd == 0),
                            stop=(kd == KT - 1),
                        )
            for _, b_sb, dst, t, psums in pair:
                qk_stage(dst, t, b_sb, psums)
            w_prefetch(3)
            w_prefetch(4)

            for i, (w, b_sb, dst, t) in enumerate(seq):
                if i < 2:
                    continue
                w_sb = w_tiles.pop(i)
                w_prefetch(i + 3)
                psums = [aps.tile([128, 512], F32, tag="apsum", name=f"qk{sc}")
                         for sc in range(4)]
                for kd in range(KT):
                    for sc in range(4):
                        nc.tensor.matmul(
                            psums[sc],
                            w_sb[:, kd, :],
                            x_all[:, kd, sc * 512 : (sc + 1) * 512],
                            start=(kd == 0),
                            stop=(kd == KT - 1),
                        )
                qk_stage(dst, t, b_sb, psums)

            # V: per s-tile: psum[s 128, dh 512] x2 halves over 16 kd
            for si in range(ST):
                ps = [aps.tile([128, 512], F32, tag="apsum", name=f"v{si % 2}{half}")
                      for half in range(2)]
                for kd in range(KT):
                    for half in range(2):
                        nc.tensor.matmul(
                            ps[half],
                            x_all[:, kd, si * 128 : (si + 1) * 128],
                            wv_sb[:, kd, half * 512 : (half + 1) * 512],
                            start=(kd == 0),
                            stop=(kd == KT - 1),
                        )
                # no bias: P@(V+1(x)bv) folds to +bv@Wo, added host-side to bo
                for half in range(2):
                    nc.scalar.activation(
                        out=v_all[:, si, half * 512 : (half + 1) * 512],
                        in_=ps[half],
                        func=COPY,
                    )

        # ---------------- Phase B + C interleaved ----------------------------
        with (
            tc.tile_pool(name="wop", bufs=1) as wop,
            tc.tile_pool(name="bconst", bufs=1) as bcp,
            tc.tile_pool(name="ct", bufs=4) as ctpool,
            tc.tile_pool(name="ptile", bufs=6) as ppool,
            tc.tile_pool(name="msk", bufs=4) as mpool,
            tc.tile_pool(name="den", bufs=4) as dpool,
            tc.tile_pool(name="rcp", bufs=2) as rcpool,
            tc.tile_pool(name="ostage", bufs=4) as ost,
            tc.tile_pool(name="pscore", bufs=3, space="PSUM") as pscore,
            tc.tile_pool(name="pctx", bufs=3, space="PSUM") as pctx,
            tc.tile_pool(name="opsum", bufs=2, space="PSUM") as ops,
        ):
            cm_sb = bcp.tile([128, 896], F32)
            nc.sync.dma_start(out=cm_sb, in_=cmask[:, :])
            ones_bf = bcp.tile([128, 128], BF16)
            nc.vector.memset(ones_bf, 1.0)
            wo_sb = wop.tile([128, HPC, D], BF16)
            wo_r = wo.rearrange("(n p) m -> p n m", p=128)
            nc.sync.dma_start(out=wo_sb[:, 0:4, :], in_=wo_r[:, 0:4, :])
            nc.scalar.dma_start(out=wo_sb[:, 4:8, :], in_=wo_r[:, 4:8, :])

            ct_chunks = {}

            def phase_b(qc):
                nkt = 4 * qc + 4
                ct_chunk = ctpool.tile([128, HPC, 512], BF16, tag="ct",
                                       name=f"ct{qc % 2}")
                ct_chunks[qc] = ct_chunk
                use_pe_den = qc < 2
                for h in range(HPC):
                    # diagonal tiles first: longer PE->DVE->ACT chains start
                    # early and overlap with the full tiles' stream
                    order = list(range(4 * qc, nkt)) + list(range(4 * qc))
                    psum_c = pctx.tile([128, 512], F32, tag="pctx")
                    psum_den = pctx.tile([128, 512], F32, tag="pctx", name="psum_den")
                    hd = {}

                    def scores(kt_i):
                        j = kt_i - 4 * qc
                        off = 128 * j if j > 0 else 0
                        ps_t = pscore.tile([128, 512], F32, tag="ps_t")
                        nc.tensor.matmul(
                            ps_t[:, off:],
                            kt_all[:, h, kt_i * 128 : (kt_i + 1) * 128],
                            qt_all[:, h, qc * 512 + off : (qc + 1) * 512],
                            start=True,
                            stop=True,
                        )
                        p_t = ppool.tile([128, 512], BF16, tag="p_t")
                        if j >= 0:
                            msk = mpool.tile([128, 512], F32, tag="msk")
                            nc.vector.tensor_tensor(
                                out=msk[:, off:],
                                in0=ps_t[:, off:],
                                in1=cm_sb[:, 384 : 896 - off],
                                op=ADD,
                            )
                            src = msk
                        else:
                            src = ps_t
                        nc.scalar.activation(
                            out=p_t[:, off:],
                            in_=src[:, off:],
                            func=EXP,
                            scale=float(SCALE),
                        )
                        # denominator: PE ones-matmul accumulation for the
                        # PE-light early chunks, DVE add-chains for qc>=2
                        if use_pe_den:
                            nc.tensor.matmul(
                                psum_den[:, off:],
                                ones_bf,
                                p_t[:, off:],
                                start=(kt_i == order[0]),
                                stop=(kt_i == order[-1]),
                            )
                        elif j == 0:
                            hd["d0"] = p_t
                        elif j == 1:
                            den = dpool.tile([128, 512], BF16, tag="den")
                            hd["den"] = den
                            nc.vector.tensor_copy(
                                out=den[:, 0:128], in_=hd["d0"][:, 0:128]
                            )
                            nc.vector.tensor_tensor(
                                out=den[:, 128:], in0=hd["d0"][:, 128:],
                                in1=p_t[:, 128:], op=ADD,
                            )
                        elif j > 1:
                            den = hd["den"]
                            nc.vector.tensor_tensor(
                                out=den[:, off:], in0=den[:, off:],
                                in1=p_t[:, off:], op=ADD,
                            )
                        else:
                            # full tiles: two interleaved chains so the
                            # head-end latency is halved
                            nf = hd.get("nf", 0)
                            hd["nf"] = nf + 1
                            if nf == 0:
                                hd["pf0"] = p_t
                            elif nf == 1:
                                den2 = dpool.tile([128, 512], BF16, tag="den",
                                                  name="den2")
                                hd["den2"] = den2
                                nc.vector.tensor_tensor(
                                    out=den2, in0=hd["pf0"], in1=p_t, op=ADD,
                                )
                            else:
                                tgt = hd["den"] if nf % 2 == 0 else hd["den2"]
                                nc.vector.tensor_tensor(
                                    out=tgt, in0=tgt, in1=p_t, op=ADD,
                                )
                        return p_t, off

                    def ctx(idx, kt_i, p_t, off):
                        nc.tensor.matmul(
                            psum_c[:, off:],
                            v_all[:, kt_i, h * 128 : (h + 1) * 128],
                            p_t[:, off:],
                            start=(idx == 0),
                            stop=(idx == nkt - 1),
                        )

                    # software-pipeline scores/exp ahead of ctx by two tiles
                    # so the PE never waits on the DVE-mask -> ACT-exp chain
                    pend = []
                    ctx_i = 0
                    for kt_i in order:
                        p_t, off = scores(kt_i)
                        pend.append((kt_i, p_t, off))
                        if len(pend) > 2:
                            ctx(ctx_i, *pend.pop(0))
                            ctx_i += 1
                    for ent in pend:
                        ctx(ctx_i, *ent)
                        ctx_i += 1

                    if not use_pe_den:
                        if "den2" in hd:
                            nc.vector.tensor_tensor(
                                out=hd["den"], in0=hd["den"], in1=hd["den2"],
                                op=ADD,
                            )
                        nc.tensor.matmul(
                            psum_den, ones_bf, hd["den"], start=True, stop=True
                        )
                    recip = rcpool.tile([128, 512], F32, tag="rcp")
                    nc.vector.reciprocal_approx_fast(out=recip, in_=psum_den)
                    nc.vector.tensor_tensor(
                        out=ct_chunk[:, h, :],
                        in0=psum_c,
                        in1=recip,
                        op=MULT,
                    )

            def phase_c(qc):
                ct_chunk = ct_chunks.pop(qc)
                for st4 in range(4):
                    st = qc * 4 + st4
                    for ncol in range(4):
                        psum_o = ops.tile([128, 512], F32, tag="opsum")
                        for hh in range(HPC):
                            nc.tensor.matmul(
                                psum_o,
                                ct_chunk[:, hh, st4 * 128 : (st4 + 1) * 128],
                                wo_sb[:, hh, ncol * 512 : (ncol + 1) * 512],
                                start=(hh == 0),
                                stop=(hh == HPC - 1),
                            )
                        o_sb = ost.tile([128, 512], BF16, tag="ostage")
                        if (st4 + ncol) % 2 == 0:
                            nc.vector.tensor_copy(out=o_sb, in_=psum_o)
                        else:
                            nc.scalar.activation(out=o_sb, in_=psum_o, func=COPY)
                        nc.gpsimd.dma_start(
                            out=out[
                                st * 128 : (st + 1) * 128,
                                ncol * 512 : (ncol + 1) * 512,
                            ],
                            in_=o_sb,
                        )

            # emission order: C(qc) lands 2 chunks later so the PE queue stays
            # full while the DVE grinds the mask/denominator chains of the
            # small early chunks
            phase_b(0)
            phase_b(1)
            phase_b(2)
            phase_c(0)
            phase_b(3)
            phase_c(1)
            phase_c(2)
            phase_c(3)


_NC = None


def _get_nc():
    global _NC
    if _NC is None:
        _NC = _build_nc()
    return _NC


def _host_prep(input_sequences, Wq, bq, Wk, bk, Wv, bv, Wo, bo):
    """Build per-core input maps (all bf16 except biases/mask)."""
    x = np.asarray(input_sequences, dtype=np.float32)
    cm = np.full((128, 896), NEG, dtype=np.float32)
    kk = np.arange(128)[:, None]
    uu = np.arange(896)[None, :]
    cm[kk <= uu - 384] = 0.0

    xT_b = [np.ascontiguousarray(x[b].T).astype(BF) for b in range(B)]
    halves = []
    for g in range(2):
        sl = slice(g * DHG, (g + 1) * DHG)
        wq_c = np.ascontiguousarray(
            np.asarray(Wq[:, sl], dtype=np.float32)
            .reshape(KT, 128, HPC, 128).transpose(2, 1, 0, 3).reshape(DHG, D)
        ).astype(BF)
        wk_c = np.ascontiguousarray(
            np.asarray(Wk[:, sl], dtype=np.float32)
            .reshape(KT, 128, HPC, 128).transpose(2, 1, 0, 3).reshape(DHG, D)
        ).astype(BF)
        wv_c = np.ascontiguousarray(Wv[:, sl]).astype(BF)
        wo_c = np.ascontiguousarray(Wo[sl, :]).astype(BF)
        halves.append({
            "wq": wq_c,
            "wk": wk_c,
            "wv": wv_c,
            "wo": wo_c,
            "bqT": np.ascontiguousarray(
                np.asarray(bq[sl], dtype=np.float32).reshape(HPC, 128).T
            ),
            "bkT": np.ascontiguousarray(
                np.asarray(bk[sl], dtype=np.float32).reshape(HPC, 128).T
            ),
            "cmask": cm,
        })

    in_maps = []
    for c in range(8):
        b, g = divmod(c, 2)
        in_maps.append({"xT": xT_b[b], **halves[g]})
    return in_maps


def kernel(input_sequences, Wq, bq, Wk, bk, Wv, bv, Wo, bo, _trace=False):
    nc = _get_nc()
    in_maps = _host_prep(input_sequences, Wq, bq, Wk, bk, Wv, bv, Wo, bo)
    res = run_bass_kernel_spmd(nc, in_maps, list(range(8)), trace=_trace)
    # the V bias is folded out of the device kernel: softmax weights sum to 1,
    # so dropping bv from V shifts every output row by exactly bv @ Wo
    bo_eff = (
        np.asarray(bo, dtype=np.float32)
        + np.asarray(bv, dtype=np.float32) @ np.asarray(Wo, dtype=np.float32)
    )
    out = np.empty((B, S, D), dtype=np.float32)
    for b in range(B):
        out[b] = (
            res.results[2 * b]["out"].astype(np.float32)
            + res.results[2 * b + 1]["out"].astype(np.float32)
            + bo_eff
        )
    if _trace:
        kernel.last_exec_time_ns = res.exec_time_ns
    return out
